# revision 2
# baseline (speedup 1.0000x reference)
"""Causal self-attention (B=4, T=2048, C=1024, H=16, HD=64) on 8 TRN2 NeuronCores.

Sharding: core c handles batch b = c//2 and head-group hg = c%2 (8 of 16 heads).
Each core computes q/k/v projections for its heads (tensor-parallel weight
columns), the causal softmax over full T, attention output, and its partial
output projection (tensor-parallel weight rows). Host sums the two partial
projections per batch (+ bp) and reassembles `present` from the per-core
k^T / v outputs.

On-device layout (per core):
  scores are computed transposed: S^T[j, i] blocks (j on partitions) so the
  exp'd attention feeds the av matmul as the moving operand directly.
  av uses stationary [v | 1] (M=65): partition 64 of the psum accumulates the
  softmax denominator Z, normalized out via reciprocal + partition_broadcast.
"""

import contextlib
import math

import numpy as np
import ml_dtypes

import concourse.bass as bass
import concourse.mybir as mybir
import concourse.tile as tile
from concourse import bacc
from concourse.bass_utils import run_bass_kernel_spmd

F32 = mybir.dt.float32
BF16 = mybir.dt.bfloat16

B, T, C = 4, 2048, 1024
H, HD = 16, 64
NH = 8            # heads per core
NPAIR = NH // 2   # head pairs (128 partitions = 2 heads x 64 dims)
P = 128
NCC = C // P      # contraction chunks for the projections
NTC = T // P      # 128-row t-chunks
NTB = T // 512    # 512-col t-blocks
NEG = -30000.0
SCALE = 1.0 / math.sqrt(HD)

N_CORES = 8


def build(with_bias: bool = False, with_valid: bool = False, reps: int = 1):
    """Build + compile the SPMD per-core Bass program."""
    nc = bacc.Bacc("TRN2", target_bir_lowering=False, debug=False,
                   num_devices=N_CORES)

    d = {}
    d["xT_d"] = nc.dram_tensor("xT", [C, T], BF16, kind="ExternalInput")
    d["wq_d"] = nc.dram_tensor("wq", [C, NH * HD], BF16, kind="ExternalInput")
    d["wk_d"] = nc.dram_tensor("wk", [C, NH * HD], BF16, kind="ExternalInput")
    d["wv_d"] = nc.dram_tensor("wv", [C, NH * HD], BF16, kind="ExternalInput")
    d["wp_d"] = nc.dram_tensor("wp", [NH * HD, C], BF16, kind="ExternalInput")
    if with_bias:
        d["bqk_d"] = nc.dram_tensor("bqk", [P, NPAIR, 2], F32, kind="ExternalInput")
        d["bv_d"] = nc.dram_tensor("bv", [NH * HD], F32, kind="ExternalInput")
    if with_valid:
        d["validT_d"] = nc.dram_tensor("validT", [P, NTC], F32, kind="ExternalInput")

    d["kT_o"] = nc.dram_tensor("kT_o", [NH * HD, T], F32, kind="ExternalOutput")
    d["v_o"] = nc.dram_tensor("v_o", [T, NH * HD], F32, kind="ExternalOutput")
    d["y_o"] = nc.dram_tensor("y_o", [T, C], F32, kind="ExternalOutput")
    d["with_bias"] = with_bias
    d["with_valid"] = with_valid

    with tile.TileContext(nc) as tc:
        with contextlib.ExitStack() as stack:
            pools = {
                "const": stack.enter_context(tc.tile_pool(name="const", bufs=1)),
                "io": stack.enter_context(tc.tile_pool(name="io", bufs=3)),
                "zp": stack.enter_context(tc.tile_pool(name="zp", bufs=3)),
                "psA": stack.enter_context(
                    tc.tile_pool(name="psA", bufs=2, space="PSUM")),
                "psB": stack.enter_context(
                    tc.tile_pool(name="psB", bufs=4, space="PSUM")),
            }
            if reps > 1:
                stack.enter_context(tc.For_i(0, reps, 1))
            _emit(nc, tc, pools, d)
    nc.compile()
    return nc


def _emit(nc, tc, pools, d):
    kT_o, v_o, y_o = d["kT_o"], d["v_o"], d["y_o"]
    with_bias, with_valid = d["with_bias"], d["with_valid"]
    const, io, zp = pools["const"], pools["io"], pools["zp"]
    psA, psB = pools["psA"], pools["psB"]

    # ---- persistent staging --------------------------------------------
    wq = const.tile([P, NCC, NH * HD], BF16, name="wq_s")
    nc.sync.dma_start(wq[:], d["wq_d"].rearrange("(c p) m -> p c m", p=P))
    wk = const.tile([P, NCC, NH * HD], BF16, name="wk_s")
    nc.sync.dma_start(wk[:], d["wk_d"].rearrange("(c p) m -> p c m", p=P))
    wv = const.tile([P, NCC, NH * HD], BF16, name="wv_s")
    nc.sync.dma_start(wv[:], d["wv_d"].rearrange("(c p) m -> p c m", p=P))
    wp = const.tile([P, NH * HD // P, C], BF16, name="wp_s")
    nc.sync.dma_start(wp[:], d["wp_d"].rearrange("(k p) n -> p k n", p=P))
    if with_bias:
        bqk = const.tile([P, NPAIR, 2], F32, name="bqk_s")
        nc.sync.dma_start(bqk[:], d["bqk_d"][:])
        bvb = const.tile([P, NH * HD], F32, name="bvb_s")
        nc.gpsimd.dma_start(
            bvb[:],
            bass.AP(tensor=d["bv_d"].ap().tensor, offset=0,
                    ap=[[0, P], [1, NH * HD]]),
        )
    if with_valid:
        validT = const.tile([P, NTC], F32, name="validT_s")
        nc.sync.dma_start(validT[:], d["validT_d"][:])

    # additive causal mask for the diagonal 128x128 block in S^T layout:
    # keep (0) where j_rel <= i_rel  i.e.  -p + f >= 0 ; NEG below.
    msk = const.tile([P, P], F32, name="msk_s")
    nc.gpsimd.memset(msk[:], 0.0)
    nc.gpsimd.affine_select(
        out=msk[:], in_=msk[:], compare_op=mybir.AluOpType.is_ge,
        fill=NEG, base=0, pattern=[[1, P]], channel_multiplier=-1,
    )

    qT = const.tile([P, NPAIR, T], BF16, name="qT_s")
    kT = const.tile([P, NPAIR, T], BF16, name="kT_s")
    vv = const.tile([P, NTC, NH, HD + 1], BF16, name="vv_s")
    nc.vector.memset(vv[:, :, :, HD:HD + 1], 1.0)
    yt = const.tile([P, NPAIR, T], BF16, name="yt_s")

    # ---- P1: projections (xT lives in its own pool, freed afterwards) --
    with tc.tile_pool(name="xp", bufs=1) as xp:
        xT = xp.tile([P, NCC, T], BF16, name="xT_s")
        nc.sync.dma_start(xT[:], d["xT_d"].rearrange("(c p) t -> p c t", p=P))

        # q/k in transposed layout
        for pair in range(NPAIR):
            for which, w_sb, dst in (("q", wq, qT), ("k", wk, kT)):
                if which == "q":
                    ps_tiles = [
                        psA.tile([P, 1024], F32, tag="s", name=f"pq{pair}{i}")
                        for i in range(2)
                    ]
                    seg = 1024
                else:
                    ps_tiles = [
                        psB.tile([P, 512], F32, tag="b", name=f"pk{pair}{i}")
                        for i in range(4)
                    ]
                    seg = 512
                for kc in range(NCC):
                    for tb in range(NTB):
                        ps = ps_tiles[tb * 512 // seg]
                        off = (tb * 512) % seg
                        nc.tensor.matmul(
                            ps[:, off:off + 512],
                            w_sb[:, kc, pair * P:(pair + 1) * P],
                            xT[:, kc, tb * 512:(tb + 1) * 512],
                            start=(kc == 0), stop=(kc == NCC - 1),
                        )
                for tb in range(NTB):
                    ps = ps_tiles[tb * 512 // seg]
                    off = (tb * 512) % seg
                    src = ps[:, off:off + 512]
                    if with_bias:
                        bcol = 0 if which == "q" else 1
                        nc.vector.tensor_scalar_add(
                            src, src, bqk[:, pair, bcol:bcol + 1])
                    nc.scalar.copy(
                        out=dst[:, pair, tb * 512:(tb + 1) * 512], in_=src)
                    if which == "k":
                        st = io.tile([P, 512], F32, tag="io", name=f"sk{pair}{tb}")
                        nc.vector.tensor_copy(st[:], src)
                        nc.sync.dma_start(
                            kT_o[pair * P:(pair + 1) * P,
                                 tb * 512:(tb + 1) * 512],
                            st[:],
                        )

        # v in natural layout (with implicit ones column in vv)
        for tch in range(NTC):
            psv = psB.tile([P, 512], F32, tag="b", name=f"pv{tch}")
            for kc in range(NCC):
                nc.tensor.matmul(
                    psv[:],
                    xT[:, kc, tch * P:(tch + 1) * P],
                    wv[:, kc, :],
                    start=(kc == 0), stop=(kc == NCC - 1),
                )
            if with_bias:
                nc.vector.tensor_add(psv[:], psv[:], bvb[:])
            nc.scalar.copy(
                out=vv[:, tch, :, 0:HD],
                in_=psv[:].rearrange("p (h d) -> p h d", h=NH),
            )
            stv = io.tile([P, 512], F32, tag="io", name=f"sv{tch}")
            nc.vector.tensor_copy(stv[:], psv[:])
            nc.sync.dma_start(v_o[tch * P:(tch + 1) * P, :], stv[:])

    # ---- P2: attention, one head at a time -----------------------------
    with tc.tile_pool(name="attp", bufs=17) as attp:
        for h in range(NH):
            pair, half = h // 2, h % 2
            lo, hi = half * HD, (half + 1) * HD
            yps = [None] * NTB
            for jb in range(NTC):
                j0 = jb * P
                strip = attp.tile([P, T], BF16, tag="att", name=f"at{h}_{jb}")
                pieces = ([(j0, 1024), (1024, 2048)] if j0 < 1024
                          else [(j0, 2048)])
                for (pa, pb) in pieces:
                    w = pb - pa
                    ps = psA.tile([P, 1024], F32, tag="s",
                                  name=f"ps{h}_{jb}_{pa}")
                    off = 0
                    while off < w:
                        n = min(512, w - off)
                        nc.tensor.matmul(
                            ps[:, off:off + n],
                            kT[lo:hi, pair, j0:j0 + P],
                            qT[lo:hi, pair, pa + off:pa + off + n],
                            start=True, stop=True,
                        )
                        off += n
                    if pa == j0:
                        nc.vector.tensor_add(ps[:, 0:P], ps[:, 0:P], msk[:])
                    nc.scalar.activation(
                        out=strip[:, pa:pb], in_=ps[:, 0:w],
                        func=mybir.ActivationFunctionType.Exp, scale=SCALE,
                    )
                    if with_valid:
                        nc.vector.tensor_scalar_mul(
                            strip[:, pa:pb], strip[:, pa:pb],
                            validT[:, jb:jb + 1])
                # av accumulation; rhs ragged at the diagonal
                for ib in range(j0 // 512, NTB):
                    if yps[ib] is None:
                        yps[ib] = psB.tile([HD + 1, 512], F32, tag="b",
                                           name=f"yp{h}_{ib}")
                    yp = yps[ib]
                    i_lo = max(ib * 512, j0)
                    ooff = i_lo - ib * 512
                    last_jb = 4 * (ib + 1) - 1
                    nc.tensor.matmul(
                        yp[:, ooff:512],
                        vv[:, jb, h, :],
                        strip[:, i_lo:(ib + 1) * 512],
                        start=(jb == 0), stop=(jb == last_jb),
                    )
                    if jb == last_jb:
                        zr = zp.tile([1, 512], F32, tag="zr", name=f"zr{h}_{ib}")
                        nc.vector.reciprocal(zr[:], yp[HD:HD + 1, :])
                        zb = zp.tile([HD, 512], F32, tag="zb", name=f"zb{h}_{ib}")
                        nc.gpsimd.partition_broadcast(zb[:], zr[:])
                        nc.vector.tensor_mul(
                            yt[lo:hi, pair, ib * 512:(ib + 1) * 512],
                            yp[0:HD, :], zb[:],
                        )
                        yps[ib] = None

    # ---- P3: partial output projection ---------------------------------
    for tch in range(NTC):
        for cb in range(2):
            psp = psB.tile([P, 512], F32, tag="b", name=f"pp{tch}_{cb}")
            for kc in range(NH * HD // P):
                nc.tensor.matmul(
                    psp[:],
                    yt[:, kc, tch * P:(tch + 1) * P],
                    wp[:, kc, cb * 512:(cb + 1) * 512],
                    start=(kc == 0), stop=(kc == NH * HD // P - 1),
                )
            stp = io.tile([P, 512], F32, tag="io", name=f"sp{tch}_{cb}")
            nc.vector.tensor_copy(stp[:], psp[:])
            nc.sync.dma_start(
                y_o[tch * P:(tch + 1) * P, cb * 512:(cb + 1) * 512], stp[:]
            )


_CACHE = {}


def get_nc(with_bias, with_valid, reps=1):
    key = (with_bias, with_valid, reps)
    if key not in _CACHE:
        _CACHE[key] = build(with_bias, with_valid, reps)
    return _CACHE[key]


def make_in_maps(x, valid_mask, Wq, bq, Wk, bk, Wv, bv, Wp, bp,
                 with_bias, with_valid):
    bf = ml_dtypes.bfloat16
    in_maps = []
    for core in range(N_CORES):
        b, hg = core // 2, core % 2
        cols = slice(hg * NH * HD, (hg + 1) * NH * HD)
        m = {
            "xT": np.ascontiguousarray(x[b].T).astype(bf),
            "wq": np.ascontiguousarray(Wq[:, cols]).astype(bf),
            "wk": np.ascontiguousarray(Wk[:, cols]).astype(bf),
            "wv": np.ascontiguousarray(Wv[:, cols]).astype(bf),
            "wp": np.ascontiguousarray(Wp[cols, :]).astype(bf),
        }
        if with_bias:
            bq_c = bq[cols].reshape(NPAIR, 2 * HD).T      # [128, NPAIR]
            bk_c = bk[cols].reshape(NPAIR, 2 * HD).T
            m["bqk"] = np.ascontiguousarray(
                np.stack([bq_c, bk_c], axis=-1)).astype(np.float32)
            m["bv"] = np.ascontiguousarray(bv[cols]).astype(np.float32)
        if with_valid:
            vT = valid_mask[b].astype(np.float32).reshape(NTC, P).T
            m["validT"] = np.ascontiguousarray(vT)
        in_maps.append(m)
    return in_maps


def assemble(results, bp):
    y = np.empty((B, T, C), dtype=np.float32)
    present = np.empty((2, B, H, T, HD), dtype=np.float32)
    for core in range(N_CORES):
        b, hg = core // 2, core % 2
        r = results[core]
        if hg == 0:
            y[b] = r["y_o"]
        else:
            y[b] += r["y_o"]
        hsl = slice(hg * NH, (hg + 1) * NH)
        # kT_o rows: pair*128 + half*64 + d  ->  head h = 2*pair + half
        kk = np.asarray(r["kT_o"]).reshape(NPAIR, 2, HD, T)
        present[0, b, hsl] = kk.transpose(0, 1, 3, 2).reshape(NH, T, HD)
        present[1, b, hsl] = np.asarray(r["v_o"]).reshape(T, NH, HD).transpose(1, 0, 2)
    y += bp.astype(np.float32)[None, None, :]
    return y, present


def kernel(x, valid_mask, Wq, bq, Wk, bk, Wv, bv, Wp, bp, **run_kwargs):
    x = np.asarray(x, dtype=np.float32)
    valid_mask = np.asarray(valid_mask)
    Wq, Wk, Wv, Wp = (np.asarray(a, dtype=np.float32) for a in (Wq, Wk, Wv, Wp))
    bq, bk, bv, bp = (np.asarray(a, dtype=np.float32) for a in (bq, bk, bv, bp))

    with_bias = bool(np.any(bq) or np.any(bk) or np.any(bv))
    with_valid = not bool(np.all(valid_mask))

    nc = get_nc(with_bias, with_valid)
    in_maps = make_in_maps(x, valid_mask, Wq, bq, Wk, bk, Wv, bv, Wp, bp,
                           with_bias, with_valid)
    res = run_bass_kernel_spmd(nc, in_maps, list(range(N_CORES)), **run_kwargs)
    return assemble(res.results, bp)


# revision 16
# speedup vs baseline: 20.9073x; 20.9073x over previous
"""Causal self-attention (B=4, T=2048, C=1024, H=16, HD=64) on 8 TRN2 NeuronCores.

Sharding: core c handles batch b = c//2 and head-group hg = c%2 (8 of 16 heads).
Each core computes q/k/v projections for its heads (tensor-parallel weight
columns), the causal softmax over full T, attention output, and its partial
output projection (tensor-parallel weight rows). Host sums the two partial
projections per batch (+ bp) and reassembles `present` from the per-core
k^T / v outputs.

On-device layout (per core):
  scores are computed transposed: S^T[j, i] blocks (j on partitions) so the
  exp'd attention feeds the av matmul as the moving operand directly.
  av uses stationary [v | 1] (M=65): partition 64 of the psum accumulates the
  softmax denominator Z, normalized out via reciprocal + partition_broadcast.
"""

import contextlib
import math

import numpy as np
import ml_dtypes

import concourse.bass as bass
import concourse.mybir as mybir
import concourse.tile as tile
from concourse import bacc
from concourse.bass_utils import run_bass_kernel_spmd

F32 = mybir.dt.float32
BF16 = mybir.dt.bfloat16

B, T, C = 4, 2048, 1024
H, HD = 16, 64
NH = 8            # heads per core
NPAIR = NH // 2   # head pairs (128 partitions = 2 heads x 64 dims)
P = 128
NCC = C // P      # contraction chunks for the projections
NTC = T // P      # 128-row t-chunks
NTB = T // 512    # 512-col t-blocks
NEG = -30000.0
SCALE = 1.0 / math.sqrt(HD)

N_CORES = 8


def build(with_bias: bool = False, with_valid: bool = False, reps: int = 1,
          timing: bool = False, stages: str = "full"):
    """Build + compile the SPMD per-core Bass program.

    timing=True declares the real inputs/outputs as Internal DRAM (no host
    transfer; contents irrelevant for device timing) plus one tiny external
    output, so wall-clock differencing isolates device execution time.
    """
    nc = bacc.Bacc("TRN2", target_bir_lowering=False, debug=False,
                   num_devices=N_CORES)

    ikind = "Internal" if timing else "ExternalInput"
    okind = "Internal" if timing else "ExternalOutput"
    d = {}
    d["xT_d"] = nc.dram_tensor("xT", [C, T], BF16, kind=ikind)
    d["wq_d"] = nc.dram_tensor("wq", [C, NH * HD], BF16, kind=ikind)
    d["wk_d"] = nc.dram_tensor("wk", [C, NH * HD], BF16, kind=ikind)
    d["wv_d"] = nc.dram_tensor("wv", [C, NH * HD], BF16, kind=ikind)
    d["wp_d"] = nc.dram_tensor("wp", [NH * HD, C], BF16, kind=ikind)
    if with_bias:
        d["bqk_d"] = nc.dram_tensor("bqk", [P, NPAIR, 2], F32, kind=ikind)
        d["bv_d"] = nc.dram_tensor("bv", [NH * HD], F32, kind=ikind)
    if with_valid:
        d["validT_d"] = nc.dram_tensor("validT", [P, NTC], F32, kind=ikind)

    d["kT_o"] = nc.dram_tensor("kT_o", [NH * HD, T], F32, kind=okind)
    d["v_o"] = nc.dram_tensor("v_o", [T, NH * HD], F32, kind=okind)
    d["y_o"] = nc.dram_tensor("y_o", [T, C], F32, kind=okind)
    if timing:
        d["dummy_o"] = nc.dram_tensor("dummy_o", [P, 4], F32,
                                      kind="ExternalOutput")
    d["with_bias"] = with_bias
    d["with_valid"] = with_valid
    d["stages"] = stages

    with tile.TileContext(nc) as tc:
        with contextlib.ExitStack() as stack:
            pools = {
                "const": stack.enter_context(tc.tile_pool(name="const", bufs=1)),
                "io": stack.enter_context(tc.tile_pool(name="io", bufs=3)),
                "zp": stack.enter_context(tc.tile_pool(name="zp", bufs=3)),
                "psA": stack.enter_context(
                    tc.tile_pool(name="psA", bufs=2, space="PSUM")),
                "psB": stack.enter_context(
                    tc.tile_pool(name="psB", bufs=4, space="PSUM")),
            }
            if timing:
                # zero-fill the Internal input DRAM once so the timed loop
                # computes on well-formed values (no inf/NaN from stale HBM)
                zt = pools["const"].tile([P, T], BF16, name="zt")
                nc.vector.memset(zt[:], 0.0)
                xr0 = d["xT_d"].rearrange("(c p) t -> c p t", p=P)
                for kc in range(NCC):
                    nc.sync.dma_start(xr0[kc], zt[:])
                for w_d in (d["wq_d"], d["wk_d"], d["wv_d"]):
                    wr0 = w_d.rearrange("(c p) m -> c p m", p=P)
                    for kc in range(NCC):
                        nc.sync.dma_start(wr0[kc], zt[:, 0:NH * HD])
                wpr0 = d["wp_d"].rearrange("(k p) n -> k p n", p=P)
                for kc in range(NH * HD // P):
                    nc.sync.dma_start(wpr0[kc], zt[:, 0:C])
            if reps > 1:
                stack.enter_context(tc.For_i(0, reps, 1))
            if stages != "none":
                _emit(nc, tc, pools, d)
            if timing:
                dt = pools["io"].tile([P, 4], F32, tag="dum", name="dum")
                nc.vector.memset(dt[:], 1.0)
                nc.sync.dma_start(d["dummy_o"][:], dt[:])
    nc.compile()
    return nc


def _emit(nc, tc, pools, d):
    kT_o, v_o, y_o = d["kT_o"], d["v_o"], d["y_o"]
    with_bias, with_valid = d["with_bias"], d["with_valid"]
    const, io, zp = pools["const"], pools["io"], pools["zp"]
    psA, psB = pools["psA"], pools["psB"]

    # ---- persistent staging --------------------------------------------
    wq = const.tile([P, NCC, NH * HD], BF16, name="wq_s")
    nc.sync.dma_start(wq[:], d["wq_d"].rearrange("(c p) m -> p c m", p=P))
    if with_bias:
        bqk = const.tile([P, NPAIR, 2], F32, name="bqk_s")
        nc.sync.dma_start(bqk[:], d["bqk_d"][:])
        bvb = const.tile([P, NH * HD], F32, name="bvb_s")
        nc.gpsimd.dma_start(
            bvb[:],
            bass.AP(tensor=d["bv_d"].ap().tensor, offset=0,
                    ap=[[0, P], [1, NH * HD]]),
        )
    if with_valid:
        validT = const.tile([P, NTC], F32, name="validT_s")
        nc.sync.dma_start(validT[:], d["validT_d"][:])

    # multiplicative causal mask for the diagonal 128x128 block (S^T layout):
    # 1 where j_rel <= i_rel (keep), 0 below the diagonal.
    tri01 = const.tile([P, P], BF16, name="tri01_s")
    nc.gpsimd.memset(tri01[:], 1.0)
    nc.gpsimd.affine_select(
        out=tri01[:], in_=tri01[:], compare_op=mybir.AluOpType.is_ge,
        fill=0.0, base=0, pattern=[[1, P]], channel_multiplier=-1,
    )

    # per-pair / per-chunk tiles so Tile's dependency tracking lets the
    # attention phase start as soon as its own pair's projections are done
    qTs = [const.tile([P, T], BF16, name=f"qT{i}") for i in range(NPAIR)]
    kTs = [const.tile([P, T], BF16, name=f"kT{i}") for i in range(NPAIR)]
    vvs = [const.tile([P, NH, HD + 1], BF16, name=f"vv{i}") for i in range(NTC)]
    for tch in range(NTC):
        nc.vector.memset(vvs[tch][:, :, HD:HD + 1], 1.0)
    yt = const.tile([P, NPAIR, T], BF16, name="yt_s")

    xTs = [const.tile([P, T], BF16, name=f"xT{i}") for i in range(NCC)]
    xr = d["xT_d"].rearrange("(c p) t -> c p t", p=P)
    for kc in range(NCC):
        nc.sync.dma_start(xTs[kc][:], xr[kc])
    wk = const.tile([P, NCC, NH * HD], BF16, name="wk_s")
    nc.sync.dma_start(wk[:], d["wk_d"].rearrange("(c p) m -> p c m", p=P))
    wv = const.tile([P, NCC, NH * HD], BF16, name="wv_s")
    nc.sync.dma_start(wv[:], d["wv_d"].rearrange("(c p) m -> p c m", p=P))
    wp = const.tile([P, NH * HD // P, C], BF16, name="wp_s")
    nc.sync.dma_start(wp[:], d["wp_d"].rearrange("(k p) n -> p k n", p=P))

    # ---- P1/P2 interleaved -----------------------------------------------
    def emit_qk_pair(pair):
        for which, w_sb, dst in (("q", wq, qTs[pair]), ("k", wk, kTs[pair])):
            if which == "q":
                ps_tiles = [
                    psA.tile([P, 1024], F32, tag="s", name=f"pq{pair}{i}")
                    for i in range(2)
                ]
                seg = 1024
            else:
                ps_tiles = [
                    psB.tile([P, 512], F32, tag="b", name=f"pk{pair}{i}")
                    for i in range(4)
                ]
                seg = 512
            for kc in range(NCC):
                for tb in range(NTB):
                    ps = ps_tiles[tb * 512 // seg]
                    off = (tb * 512) % seg
                    nc.tensor.matmul(
                        ps[:, off:off + 512],
                        w_sb[:, kc, pair * P:(pair + 1) * P],
                        xTs[kc][:, tb * 512:(tb + 1) * 512],
                        start=(kc == 0), stop=(kc == NCC - 1),
                    )
            for tb in range(NTB):
                ps = ps_tiles[tb * 512 // seg]
                off = (tb * 512) % seg
                src_ = ps[:, off:off + 512]
                if with_bias:
                    bcol = 0 if which == "q" else 1
                    nc.vector.tensor_scalar_add(
                        src_, src_, bqk[:, pair, bcol:bcol + 1])
                nc.scalar.copy(
                    dst[:, tb * 512:(tb + 1) * 512], src_)
                if which == "k":
                    st = io.tile([P, 512], F32, tag="io", name=f"sk{pair}{tb}")
                    nc.scalar.copy(st[:], src_)
                    nc.sync.dma_start(
                        kT_o[pair * P:(pair + 1) * P,
                             tb * 512:(tb + 1) * 512],
                        st[:],
                    )

    def emit_v_chunk(tch):
        psv = psB.tile([P, 512], F32, tag="b", name=f"pv{tch}")
        for kc in range(NCC):
            nc.tensor.matmul(
                psv[:],
                xTs[kc][:, tch * P:(tch + 1) * P],
                wv[:, kc, :],
                start=(kc == 0), stop=(kc == NCC - 1),
            )
        if with_bias:
            nc.vector.tensor_add(psv[:], psv[:], bvb[:])
        nc.scalar.copy(
            vvs[tch][:, :, 0:HD],
            psv[:].rearrange("p (h d) -> p h d", h=NH),
        )
        stv = io.tile([P, 512], F32, tag="io", name=f"sv{tch}")
        nc.scalar.copy(stv[:], psv[:])
        nc.sync.dma_start(v_o[tch * P:(tch + 1) * P, :], stv[:])

    def emit_qk_block(pair, win):
        which, tb = ("q", win) if win < 4 else ("k", win - 4)
        w_sb = wq if which == "q" else wk
        dst = qTs[pair] if which == "q" else kTs[pair]
        ps = psB.tile([P, 512], F32, tag="b", name=f"pf{which}{pair}{tb}")
        for kc in range(NCC):
            nc.tensor.matmul(
                ps[:],
                w_sb[:, kc, pair * P:(pair + 1) * P],
                xTs[kc][:, tb * 512:(tb + 1) * 512],
                start=(kc == 0), stop=(kc == NCC - 1),
            )
        src_ = ps[:]
        if with_bias:
            bcol = 0 if which == "q" else 1
            nc.vector.tensor_scalar_add(
                src_, src_, bqk[:, pair, bcol:bcol + 1])
        nc.vector.tensor_copy(dst[:, tb * 512:(tb + 1) * 512], src_)
        if which == "k":
            st = io.tile([P, 512], F32, tag="io", name=f"fk{pair}{tb}")
            nc.vector.tensor_copy(st[:], src_)
            nc.sync.dma_start(
                kT_o[pair * P:(pair + 1) * P, tb * 512:(tb + 1) * 512],
                st[:],
            )

    emit_qk_pair(0)
    for _tch in range(NTC):
        emit_v_chunk(_tch)
    if d["stages"] == "p1":
        for _pr in range(1, NPAIR):
            emit_qk_pair(_pr)
        return
    # ---- P2: attention, one head at a time -----------------------------
    with tc.tile_pool(name="attp", bufs=4) as attp:
        for h in range(NH):
            pair, half = h // 2, h % 2
            lo, hi = half * HD, (half + 1) * HD
            yps = [None] * NTB
            for jb in range(NTC):
                j0 = jb * P
                strip = attp.tile([P, T], BF16, tag="att", name=f"at{h}_{jb}")
                pieces = ([(j0, 1024), (1024, 2048)] if j0 < 1024
                          else [(j0, 2048)])
                for (pa, pb) in pieces:
                    w = pb - pa
                    ps = psA.tile([P, 1024], F32, tag="s",
                                  name=f"ps{h}_{jb}_{pa}")
                    off = 0
                    while off < w:
                        n = min(512, w - off)
                        nc.tensor.matmul(
                            ps[:, off:off + n],
                            kTs[pair][lo:hi, j0:j0 + P],
                            qTs[pair][lo:hi, pa + off:pa + off + n],
                            start=True, stop=True,
                        )
                        off += n
                    nc.scalar.activation(
                        out=strip[:, pa:pb], in_=ps[:, 0:w],
                        func=mybir.ActivationFunctionType.Exp, scale=SCALE,
                    )
                    if with_valid:
                        nc.vector.tensor_scalar_mul(
                            strip[:, pa:pb], strip[:, pa:pb],
                            validT[:, jb:jb + 1])
                nc.vector.tensor_mul(
                    strip[:, j0:j0 + P], strip[:, j0:j0 + P], tri01[:])
                # av accumulation; rhs ragged at the diagonal
                for ib in range(j0 // 512, NTB):
                    if yps[ib] is None:
                        yps[ib] = psB.tile([HD + 1, 512], F32, tag="b",
                                           name=f"yp{h}_{ib}")
                    yp = yps[ib]
                    i_lo = max(ib * 512, j0)
                    ooff = i_lo - ib * 512
                    last_jb = 4 * (ib + 1) - 1
                    nc.tensor.matmul(
                        yp[:, ooff:512],
                        vvs[jb][:, h, :],
                        strip[:, i_lo:(ib + 1) * 512],
                        start=(jb == 0), stop=(jb == last_jb),
                    )
                    if jb == last_jb:
                        zr = zp.tile([1, 512], F32, tag="zr", name=f"zr{h}_{ib}")
                        nc.vector.reciprocal(zr[:], yp[HD:HD + 1, :])
                        zb = zp.tile([HD, 512], F32, tag="zb", name=f"zb{h}_{ib}")
                        nc.gpsimd.partition_broadcast(zb[:], zr[:])
                        nc.vector.tensor_mul(
                            yt[lo:hi, pair, ib * 512:(ib + 1) * 512],
                            yp[0:HD, :], zb[:],
                        )
                        yps[ib] = None
                        if pair + 1 < NPAIR:
                            emit_qk_block(pair + 1, half * 4 + ib)

    # ---- P3: partial output projection ---------------------------------
    if d["stages"] == "p1p2":
        return
    for tch in range(NTC):
        for cb in range(2):
            psp = psB.tile([P, 512], F32, tag="b", name=f"pp{tch}_{cb}")
            for kc in range(NH * HD // P):
                nc.tensor.matmul(
                    psp[:],
                    yt[:, kc, tch * P:(tch + 1) * P],
                    wp[:, kc, cb * 512:(cb + 1) * 512],
                    start=(kc == 0), stop=(kc == NH * HD // P - 1),
                )
            stp = io.tile([P, 512], F32, tag="io", name=f"sp{tch}_{cb}")
            nc.vector.tensor_copy(stp[:], psp[:])
            nc.sync.dma_start(
                y_o[tch * P:(tch + 1) * P, cb * 512:(cb + 1) * 512], stp[:]
            )


_CACHE = {}


def get_nc(with_bias, with_valid, reps=1):
    key = (with_bias, with_valid, reps)
    if key not in _CACHE:
        _CACHE[key] = build(with_bias, with_valid, reps)
    return _CACHE[key]


def make_in_maps(x, valid_mask, Wq, bq, Wk, bk, Wv, bv, Wp, bp,
                 with_bias, with_valid):
    bf = ml_dtypes.bfloat16
    in_maps = []
    for core in range(N_CORES):
        b, hg = core // 2, core % 2
        cols = slice(hg * NH * HD, (hg + 1) * NH * HD)
        m = {
            "xT": np.ascontiguousarray(x[b].T).astype(bf),
            "wq": np.ascontiguousarray(Wq[:, cols]).astype(bf),
            "wk": np.ascontiguousarray(Wk[:, cols]).astype(bf),
            "wv": np.ascontiguousarray(Wv[:, cols]).astype(bf),
            "wp": np.ascontiguousarray(Wp[cols, :]).astype(bf),
        }
        if with_bias:
            bq_c = bq[cols].reshape(NPAIR, 2 * HD).T      # [128, NPAIR]
            bk_c = bk[cols].reshape(NPAIR, 2 * HD).T
            m["bqk"] = np.ascontiguousarray(
                np.stack([bq_c, bk_c], axis=-1)).astype(np.float32)
            m["bv"] = np.ascontiguousarray(bv[cols]).astype(np.float32)
        if with_valid:
            vT = valid_mask[b].astype(np.float32).reshape(NTC, P).T
            m["validT"] = np.ascontiguousarray(vT)
        in_maps.append(m)
    return in_maps


def assemble(results, bp):
    y = np.empty((B, T, C), dtype=np.float32)
    present = np.empty((2, B, H, T, HD), dtype=np.float32)
    for core in range(N_CORES):
        b, hg = core // 2, core % 2
        r = results[core]
        if hg == 0:
            y[b] = r["y_o"]
        else:
            y[b] += r["y_o"]
        hsl = slice(hg * NH, (hg + 1) * NH)
        # kT_o rows: pair*128 + half*64 + d  ->  head h = 2*pair + half
        kk = np.asarray(r["kT_o"]).reshape(NPAIR, 2, HD, T)
        present[0, b, hsl] = kk.transpose(0, 1, 3, 2).reshape(NH, T, HD)
        present[1, b, hsl] = np.asarray(r["v_o"]).reshape(T, NH, HD).transpose(1, 0, 2)
    y += bp.astype(np.float32)[None, None, :]
    return y, present


def kernel(x, valid_mask, Wq, bq, Wk, bk, Wv, bv, Wp, bp, **run_kwargs):
    x = np.asarray(x, dtype=np.float32)
    valid_mask = np.asarray(valid_mask)
    Wq, Wk, Wv, Wp = (np.asarray(a, dtype=np.float32) for a in (Wq, Wk, Wv, Wp))
    bq, bk, bv, bp = (np.asarray(a, dtype=np.float32) for a in (bq, bk, bv, bp))

    with_bias = bool(np.any(bq) or np.any(bk) or np.any(bv))
    with_valid = not bool(np.all(valid_mask))

    nc = get_nc(with_bias, with_valid)
    in_maps = make_in_maps(x, valid_mask, Wq, bq, Wk, bk, Wv, bv, Wp, bp,
                           with_bias, with_valid)
    res = run_bass_kernel_spmd(nc, in_maps, list(range(N_CORES)), **run_kwargs)
    return assemble(res.results, bp)


# revision 19
# speedup vs baseline: 21.3912x; 1.0231x over previous
"""Causal self-attention (B=4, T=2048, C=1024, H=16, HD=64) on 8 TRN2 NeuronCores.

Sharding: core c handles batch b = c//2 and head-group hg = c%2 (8 of 16 heads).
Each core computes q/k/v projections for its heads (tensor-parallel weight
columns), the causal softmax over full T, attention output, and its partial
output projection (tensor-parallel weight rows). Host sums the two partial
projections per batch (+ bp) and reassembles `present` from the per-core
k^T / v outputs.

On-device layout (per core):
  scores are computed transposed: S^T[j, i] blocks (j on partitions) so the
  exp'd attention feeds the av matmul as the moving operand directly.
  av uses stationary [v | 1] (M=65): partition 64 of the psum accumulates the
  softmax denominator Z, normalized out via reciprocal + partition_broadcast.
"""

import contextlib
import math
import time as time_mod

import numpy as np
import ml_dtypes

import concourse.bass as bass
import concourse.mybir as mybir
import concourse.tile as tile
from concourse import bacc
from concourse.bass_utils import run_bass_kernel_spmd

F32 = mybir.dt.float32
BF16 = mybir.dt.bfloat16

B, T, C = 4, 2048, 1024
H, HD = 16, 64
NH = 8            # heads per core
NPAIR = NH // 2   # head pairs (128 partitions = 2 heads x 64 dims)
P = 128
NCC = C // P      # contraction chunks for the projections
NTC = T // P      # 128-row t-chunks
NTB = T // 512    # 512-col t-blocks
NEG = -30000.0
SCALE = 1.0 / math.sqrt(HD)

N_CORES = 8


def build(with_bias: bool = False, with_valid: bool = False, reps: int = 1,
          timing: bool = False, stages: str = "full"):
    """Build + compile the SPMD per-core Bass program.

    timing=True declares the real inputs/outputs as Internal DRAM (no host
    transfer; contents irrelevant for device timing) plus one tiny external
    output, so wall-clock differencing isolates device execution time.
    """
    nc = bacc.Bacc("TRN2", target_bir_lowering=False, debug=False,
                   num_devices=N_CORES)

    ikind = "Internal" if timing else "ExternalInput"
    okind = "Internal" if timing else "ExternalOutput"
    d = {}
    d["xT_d"] = nc.dram_tensor("xT", [C, T], BF16, kind=ikind)
    d["wq_d"] = nc.dram_tensor("wq", [C, NH * HD], BF16, kind=ikind)
    d["wk_d"] = nc.dram_tensor("wk", [C, NH * HD], BF16, kind=ikind)
    d["wv_d"] = nc.dram_tensor("wv", [C, NH * HD], BF16, kind=ikind)
    d["wp_d"] = nc.dram_tensor("wp", [NH * HD, C], BF16, kind=ikind)
    if with_bias:
        d["bqk_d"] = nc.dram_tensor("bqk", [P, NPAIR, 2], F32, kind=ikind)
        d["bv_d"] = nc.dram_tensor("bv", [NH * HD], F32, kind=ikind)
    if with_valid:
        d["validT_d"] = nc.dram_tensor("validT", [P, NTC], F32, kind=ikind)

    d["kT_o"] = nc.dram_tensor("kT_o", [NH * HD, T], F32, kind=okind)
    d["v_o"] = nc.dram_tensor("v_o", [T, NH * HD], F32, kind=okind)
    d["y_o"] = nc.dram_tensor("y_o", [T, C], F32, kind=okind)
    if timing:
        d["dummy_o"] = nc.dram_tensor("dummy_o", [P, 4], F32,
                                      kind="ExternalOutput")
    d["with_bias"] = with_bias
    d["with_valid"] = with_valid
    d["stages"] = stages

    with tile.TileContext(nc) as tc:
        with contextlib.ExitStack() as stack:
            pools = {
                "const": stack.enter_context(tc.tile_pool(name="const", bufs=1)),
                "io": stack.enter_context(tc.tile_pool(name="io", bufs=3)),
                "zp": stack.enter_context(tc.tile_pool(name="zp", bufs=3)),
                "psA": stack.enter_context(
                    tc.tile_pool(name="psA", bufs=2, space="PSUM")),
                "psB": stack.enter_context(
                    tc.tile_pool(name="psB", bufs=4, space="PSUM")),
            }
            if timing:
                # zero-fill the Internal input DRAM once so the timed loop
                # computes on well-formed values (no inf/NaN from stale HBM)
                zt = pools["const"].tile([P, T], BF16, name="zt")
                nc.vector.memset(zt[:], 0.0)
                xr0 = d["xT_d"].rearrange("(c p) t -> c p t", p=P)
                for kc in range(NCC):
                    nc.sync.dma_start(xr0[kc], zt[:])
                for w_d in (d["wq_d"], d["wk_d"], d["wv_d"]):
                    wr0 = w_d.rearrange("(c p) m -> c p m", p=P)
                    for kc in range(NCC):
                        nc.sync.dma_start(wr0[kc], zt[:, 0:NH * HD])
                wpr0 = d["wp_d"].rearrange("(k p) n -> k p n", p=P)
                for kc in range(NH * HD // P):
                    nc.sync.dma_start(wpr0[kc], zt[:, 0:C])
            if reps > 1:
                stack.enter_context(tc.For_i(0, reps, 1))
            if stages != "none":
                _emit(nc, tc, pools, d)
            if timing:
                dt = pools["io"].tile([P, 4], F32, tag="dum", name="dum")
                nc.vector.memset(dt[:], 1.0)
                nc.sync.dma_start(d["dummy_o"][:], dt[:])
    nc.compile()
    return nc


def _emit(nc, tc, pools, d):
    kT_o, v_o, y_o = d["kT_o"], d["v_o"], d["y_o"]
    with_bias, with_valid = d["with_bias"], d["with_valid"]
    const, io, zp = pools["const"], pools["io"], pools["zp"]
    psA, psB = pools["psA"], pools["psB"]

    # ---- persistent staging --------------------------------------------
    wq = const.tile([P, NCC, NH * HD], BF16, name="wq_s")
    nc.sync.dma_start(wq[:], d["wq_d"].rearrange("(c p) m -> p c m", p=P))
    if with_bias:
        bqk = const.tile([P, NPAIR, 2], F32, name="bqk_s")
        nc.sync.dma_start(bqk[:], d["bqk_d"][:])
        bvb = const.tile([P, NH * HD], F32, name="bvb_s")
        nc.gpsimd.dma_start(
            bvb[:],
            bass.AP(tensor=d["bv_d"].ap().tensor, offset=0,
                    ap=[[0, P], [1, NH * HD]]),
        )
    if with_valid:
        validT = const.tile([P, NTC], F32, name="validT_s")
        nc.sync.dma_start(validT[:], d["validT_d"][:])

    # multiplicative causal mask for the diagonal 128x128 block (S^T layout):
    # 1 where j_rel <= i_rel (keep), 0 below the diagonal.
    tri01 = const.tile([P, P], BF16, name="tri01_s")
    nc.gpsimd.memset(tri01[:], 1.0)
    nc.gpsimd.affine_select(
        out=tri01[:], in_=tri01[:], compare_op=mybir.AluOpType.is_ge,
        fill=0.0, base=0, pattern=[[1, P]], channel_multiplier=-1,
    )

    # per-pair / per-chunk tiles so Tile's dependency tracking lets the
    # attention phase start as soon as its own pair's projections are done
    qTs = [const.tile([P, T], BF16, name=f"qT{i}") for i in range(NPAIR)]
    kTs = [const.tile([P, T], BF16, name=f"kT{i}") for i in range(NPAIR)]
    vvs = [const.tile([P, NH, HD + 1], BF16, name=f"vv{i}") for i in range(NTC)]
    for tch in range(NTC):
        nc.vector.memset(vvs[tch][:, :, HD:HD + 1], 1.0)
    yt = const.tile([P, NPAIR, T], BF16, name="yt_s")

    xTs = [const.tile([P, T], BF16, name=f"xT{i}") for i in range(NCC)]
    xr = d["xT_d"].rearrange("(c p) t -> c p t", p=P)
    for kc in range(NCC):
        nc.sync.dma_start(xTs[kc][:], xr[kc])
    wk = const.tile([P, NCC, NH * HD], BF16, name="wk_s")
    nc.sync.dma_start(wk[:], d["wk_d"].rearrange("(c p) m -> p c m", p=P))
    wv = const.tile([P, NCC, NH * HD], BF16, name="wv_s")
    nc.sync.dma_start(wv[:], d["wv_d"].rearrange("(c p) m -> p c m", p=P))
    wp = const.tile([P, NH * HD // P, C], BF16, name="wp_s")
    nc.sync.dma_start(wp[:], d["wp_d"].rearrange("(k p) n -> p k n", p=P))

    # ---- P1/P2 interleaved -----------------------------------------------
    def emit_qk_pair(pair):
        for which, w_sb, dst in (("q", wq, qTs[pair]), ("k", wk, kTs[pair])):
            if which == "q":
                ps_tiles = [
                    psA.tile([P, 1024], F32, tag="s", name=f"pq{pair}{i}")
                    for i in range(2)
                ]
                seg = 1024
            else:
                ps_tiles = [
                    psB.tile([P, 512], F32, tag="b", name=f"pk{pair}{i}")
                    for i in range(4)
                ]
                seg = 512
            for kc in range(NCC):
                for tb in range(NTB):
                    ps = ps_tiles[tb * 512 // seg]
                    off = (tb * 512) % seg
                    nc.tensor.matmul(
                        ps[:, off:off + 512],
                        w_sb[:, kc, pair * P:(pair + 1) * P],
                        xTs[kc][:, tb * 512:(tb + 1) * 512],
                        start=(kc == 0), stop=(kc == NCC - 1),
                    )
            for tb in range(NTB):
                ps = ps_tiles[tb * 512 // seg]
                off = (tb * 512) % seg
                src_ = ps[:, off:off + 512]
                if with_bias:
                    bcol = 0 if which == "q" else 1
                    nc.vector.tensor_scalar_add(
                        src_, src_, bqk[:, pair, bcol:bcol + 1])
                nc.scalar.copy(
                    dst[:, tb * 512:(tb + 1) * 512], src_)
                if which == "k":
                    st = io.tile([P, 512], F32, tag="io", name=f"sk{pair}{tb}")
                    nc.scalar.copy(st[:], src_)
                    nc.sync.dma_start(
                        kT_o[pair * P:(pair + 1) * P,
                             tb * 512:(tb + 1) * 512],
                        st[:],
                    )

    def emit_v_chunk(tch):
        psv = psB.tile([P, 512], F32, tag="b", name=f"pv{tch}")
        for kc in range(NCC):
            nc.tensor.matmul(
                psv[:],
                xTs[kc][:, tch * P:(tch + 1) * P],
                wv[:, kc, :],
                start=(kc == 0), stop=(kc == NCC - 1),
            )
        if with_bias:
            nc.vector.tensor_add(psv[:], psv[:], bvb[:])
        nc.scalar.copy(
            vvs[tch][:, :, 0:HD],
            psv[:].rearrange("p (h d) -> p h d", h=NH),
        )
        stv = io.tile([P, 512], F32, tag="io", name=f"sv{tch}")
        nc.scalar.copy(stv[:], psv[:])
        nc.sync.dma_start(v_o[tch * P:(tch + 1) * P, :], stv[:])

    def emit_qk_block(pair, win):
        which, tb = ("q", win) if win < 4 else ("k", win - 4)
        w_sb = wq if which == "q" else wk
        dst = qTs[pair] if which == "q" else kTs[pair]
        ps = psB.tile([P, 512], F32, tag="b", name=f"pf{which}{pair}{tb}")
        for kc in range(NCC):
            nc.tensor.matmul(
                ps[:],
                w_sb[:, kc, pair * P:(pair + 1) * P],
                xTs[kc][:, tb * 512:(tb + 1) * 512],
                start=(kc == 0), stop=(kc == NCC - 1),
            )
        src_ = ps[:]
        if with_bias:
            bcol = 0 if which == "q" else 1
            nc.vector.tensor_scalar_add(
                src_, src_, bqk[:, pair, bcol:bcol + 1])
        nc.vector.tensor_copy(dst[:, tb * 512:(tb + 1) * 512], src_)
        if which == "k":
            st = io.tile([P, 512], F32, tag="io", name=f"fk{pair}{tb}")
            nc.vector.tensor_copy(st[:], src_)
            nc.sync.dma_start(
                kT_o[pair * P:(pair + 1) * P, tb * 512:(tb + 1) * 512],
                st[:],
            )

    emit_qk_pair(0)
    for _tch in range(NTC):
        emit_v_chunk(_tch)
    if d["stages"] == "p1":
        for _pr in range(1, NPAIR):
            emit_qk_pair(_pr)
        return
    # ---- P2: attention, one head at a time -----------------------------
    with tc.tile_pool(name="attp", bufs=4) as attp:
        for h in range(NH):
            pair, half = h // 2, h % 2
            lo, hi = half * HD, (half + 1) * HD
            yps = [None] * NTB
            for jb in range(NTC):
                j0 = jb * P
                strip = attp.tile([P, T], BF16, tag="att", name=f"at{h}_{jb}")
                pieces = ([(j0, 1024), (1024, 2048)] if j0 < 1024
                          else [(j0, 2048)])
                for (pa, pb) in pieces:
                    w = pb - pa
                    ps = psA.tile([P, 1024], F32, tag="s",
                                  name=f"ps{h}_{jb}_{pa}")
                    off = 0
                    while off < w:
                        n = min(512, w - off)
                        nc.tensor.matmul(
                            ps[:, off:off + n],
                            kTs[pair][lo:hi, j0:j0 + P],
                            qTs[pair][lo:hi, pa + off:pa + off + n],
                            start=True, stop=True,
                        )
                        off += n
                    nc.scalar.activation(
                        out=strip[:, pa:pb], in_=ps[:, 0:w],
                        func=mybir.ActivationFunctionType.Exp, scale=SCALE,
                    )
                    if with_valid:
                        nc.vector.tensor_scalar_mul(
                            strip[:, pa:pb], strip[:, pa:pb],
                            validT[:, jb:jb + 1])
                nc.vector.tensor_mul(
                    strip[:, j0:j0 + P], strip[:, j0:j0 + P], tri01[:])
                # av accumulation; rhs ragged at the diagonal
                for ib in range(j0 // 512, NTB):
                    if yps[ib] is None:
                        yps[ib] = psB.tile([HD + 1, 512], F32, tag="b",
                                           name=f"yp{h}_{ib}")
                    yp = yps[ib]
                    i_lo = max(ib * 512, j0)
                    ooff = i_lo - ib * 512
                    last_jb = 4 * (ib + 1) - 1
                    nc.tensor.matmul(
                        yp[:, ooff:512],
                        vvs[jb][:, h, :],
                        strip[:, i_lo:(ib + 1) * 512],
                        start=(jb == 0), stop=(jb == last_jb),
                    )
                    if jb == last_jb:
                        zr = zp.tile([1, 512], F32, tag="zr", name=f"zr{h}_{ib}")
                        nc.vector.reciprocal(zr[:], yp[HD:HD + 1, :])
                        zb = zp.tile([HD, 512], F32, tag="zb", name=f"zb{h}_{ib}")
                        nc.gpsimd.partition_broadcast(zb[:], zr[:])
                        nc.vector.tensor_mul(
                            yt[lo:hi, pair, ib * 512:(ib + 1) * 512],
                            yp[0:HD, :], zb[:],
                        )
                        yps[ib] = None
                        if pair + 1 < NPAIR:
                            emit_qk_block(pair + 1, half * 4 + ib)

    # ---- P3: partial output projection ---------------------------------
    if d["stages"] == "p1p2":
        return
    for tch in range(NTC):
        for cb in range(2):
            psp = psB.tile([P, 512], F32, tag="b", name=f"pp{tch}_{cb}")
            for kc in range(NH * HD // P):
                nc.tensor.matmul(
                    psp[:],
                    yt[:, kc, tch * P:(tch + 1) * P],
                    wp[:, kc, cb * 512:(cb + 1) * 512],
                    start=(kc == 0), stop=(kc == NH * HD // P - 1),
                )
            stp = io.tile([P, 512], F32, tag="io", name=f"sp{tch}_{cb}")
            if (tch * 2 + cb) % 2 == 0:
                nc.vector.tensor_copy(stp[:], psp[:])
            else:
                nc.scalar.copy(stp[:], psp[:])
            nc.sync.dma_start(
                y_o[tch * P:(tch + 1) * P, cb * 512:(cb + 1) * 512], stp[:]
            )


_CACHE = {}


def get_nc(with_bias, with_valid, reps=1):
    key = (with_bias, with_valid, reps)
    if key not in _CACHE:
        _CACHE[key] = build(with_bias, with_valid, reps)
    return _CACHE[key]


def make_in_maps(x, valid_mask, Wq, bq, Wk, bk, Wv, bv, Wp, bp,
                 with_bias, with_valid):
    bf = ml_dtypes.bfloat16
    in_maps = []
    for core in range(N_CORES):
        b, hg = core // 2, core % 2
        cols = slice(hg * NH * HD, (hg + 1) * NH * HD)
        m = {
            "xT": np.ascontiguousarray(x[b].T).astype(bf),
            "wq": np.ascontiguousarray(Wq[:, cols]).astype(bf),
            "wk": np.ascontiguousarray(Wk[:, cols]).astype(bf),
            "wv": np.ascontiguousarray(Wv[:, cols]).astype(bf),
            "wp": np.ascontiguousarray(Wp[cols, :]).astype(bf),
        }
        if with_bias:
            bq_c = bq[cols].reshape(NPAIR, 2 * HD).T      # [128, NPAIR]
            bk_c = bk[cols].reshape(NPAIR, 2 * HD).T
            m["bqk"] = np.ascontiguousarray(
                np.stack([bq_c, bk_c], axis=-1)).astype(np.float32)
            m["bv"] = np.ascontiguousarray(bv[cols]).astype(np.float32)
        if with_valid:
            vT = valid_mask[b].astype(np.float32).reshape(NTC, P).T
            m["validT"] = np.ascontiguousarray(vT)
        in_maps.append(m)
    return in_maps


def assemble(results, bp):
    y = np.empty((B, T, C), dtype=np.float32)
    present = np.empty((2, B, H, T, HD), dtype=np.float32)
    for core in range(N_CORES):
        b, hg = core // 2, core % 2
        r = results[core]
        if hg == 0:
            y[b] = r["y_o"]
        else:
            y[b] += r["y_o"]
        hsl = slice(hg * NH, (hg + 1) * NH)
        # kT_o rows: pair*128 + half*64 + d  ->  head h = 2*pair + half
        kk = np.asarray(r["kT_o"]).reshape(NPAIR, 2, HD, T)
        present[0, b, hsl] = kk.transpose(0, 1, 3, 2).reshape(NH, T, HD)
        present[1, b, hsl] = np.asarray(r["v_o"]).reshape(T, NH, HD).transpose(1, 0, 2)
    y += bp.astype(np.float32)[None, None, :]
    return y, present


def kernel(x, valid_mask, Wq, bq, Wk, bk, Wv, bv, Wp, bp, **run_kwargs):
    x = np.asarray(x, dtype=np.float32)
    valid_mask = np.asarray(valid_mask)
    Wq, Wk, Wv, Wp = (np.asarray(a, dtype=np.float32) for a in (Wq, Wk, Wv, Wp))
    bq, bk, bv, bp = (np.asarray(a, dtype=np.float32) for a in (bq, bk, bv, bp))

    with_bias = bool(np.any(bq) or np.any(bk) or np.any(bv))
    with_valid = not bool(np.all(valid_mask))

    nc = get_nc(with_bias, with_valid)
    in_maps = make_in_maps(x, valid_mask, Wq, bq, Wk, bk, Wv, bv, Wp, bp,
                           with_bias, with_valid)
    last_err = None
    for _attempt in range(3):
        try:
            res = run_bass_kernel_spmd(nc, in_maps, list(range(N_CORES)),
                                       **run_kwargs)
            break
        except Exception as e:  # transient device wedge: retry
            last_err = e
            time_mod.sleep(2.0)
    else:
        raise last_err
    return assemble(res.results, bp)
